# revision 6
# baseline (speedup 1.0000x reference)
"""NT-Xent loss kernel for Trainium2, SPMD across 8 NeuronCores.

Strategy (symmetric/circulant decomposition, ~half the exp work), v2:
  - Host precomputes xn = x/||x|| (f32), scales by SCALE and quantizes to
    fp8-e4m3 in the DoubleRow transposed layout [128, 2, EXTN] (k = s*128+p),
    rotated per core so the device program is identical across cores.
  - sim is symmetric: only block-distances d = (jblk - iblk) mod 64 in
    {0..32} (128-wide blocks) are computed. Core c owns m-tile rows
    t = 8*mt + c; each of its 8 windows is the contiguous circulant range
    of 33 blocks = 4224 cols. lhsT for window mt is XE[:, :, 1024mt:+128].
  - exp is split across three engines per window (ext-relative cols):
      ACT:  exp([0,1408)) and exp([1408,2944)) from PSUM tiles T0/T1 with
            accum_out rowsum parts (covers the d=0 block too).
      DVE:  bitcast-exp of T2 [2944,4224): i16 = round(psum*A + B) viewed
            as bf16 ~= exp (Schraudolph, ~+-3%), 1 instr, no LUT.
      Folds (transposed/column contributions, COL coords = 1024mt+ext-128):
        Pool: add eo[128,2048) into COL[1024mt:+1920)      (window 0: copy)
        DVE:  add eo[2048,3072) into COL[1024mt+1920:+2944) (window 0: copy)
        DVE:  first-writer copy eo[3072,4096) -> COL[1024mt+2944:+3968)
              with accum_out (rowsum part), plus scratch copies of
              [2944,3072) and the d=32 block [4096,4224) with accum_out.
    No COL memset: the window-0 copies + first-writer E45 writes cover COL.
  - Column sums: ones-matmuls on the tensor engine per 128-col chunk,
    software-pipelined one window behind the folds.
  - Outputs per core: [128, 40] rowsum partials (5 accums x 8 windows) and
    [128, 87] column-sum partials. Host: rowsum assembly (np.add.at for the
    transposed parts), loss = (sum log(rowsum) - target)/N.
"""

import sys

sys.path.insert(0, "/opt/trn_rl_repo")

from contextlib import ExitStack

import numpy as np

import concourse.bass as bass
import concourse.tile as tile
from concourse import bacc, bass_utils, mybir

F32 = mybir.dt.float32
F8 = mybir.dt.float8e4
BF16 = mybir.dt.bfloat16
I16 = mybir.dt.int16
AF = mybir.ActivationFunctionType
ALU = mybir.AluOpType
DR = mybir.MatmulPerfMode.DoubleRow

N, D = 8192, 256
NCORES = 8
SHARD = N // NCORES  # 1024 rows per core
MT = SHARD // 128  # 8 windows per core
KT = 2  # two 128-row k-subtiles (D=256), packed via DoubleRow
NBLK = 33
WIN = NBLK * 128  # 4224
EXTN = (MT - 1) * 1024 + WIN  # 11392 extended cols
COLW = (MT - 1) * 1024 + (WIN - 128) - 128  # 11136 col accumulator width
NCHUNK_COL = COLW // 128  # 87
NACC = 4 * MT  # 32 rowsum accumulator columns

TEMP = 0.5
INV_TEMP = 1.0 / TEMP
EPS = 1e-8
SCALE = 8.0  # fp8 pre-scale on xn entries
EXP_SCALE = INV_TEMP / (SCALE * SCALE)

# Schraudolph bitcast-exp: i16 = round(psum*SCH_A + SCH_B); bits viewed as
# bf16 give ~exp(psum*EXP_SCALE).  CADJ centers the sawtooth error.
LN2 = 0.6931471805599453
CADJ = 0.043
SCH_A = EXP_SCALE * 128.0 / LN2
SCH_B = 128.0 * (127.0 - CADJ)

# ext-relative region boundaries (PSUM tiles / engine ownership)
T0W, T1W = 1536, 1536  # ACT psum tiles [0,1536) [1536,3072)
DMA_W = 2432  # dma-accum fold piece: eo[128, 2560) -> COL[1024mt, +2432)

_CACHE = {}


def _build():
    nc = bacc.Bacc("TRN2", target_bir_lowering=False, debug=False, num_devices=NCORES)

    xa0 = nc.dram_tensor("xa0", [128, EXTN], F8, kind="ExternalInput").ap()
    xa1 = nc.dram_tensor("xa1", [128, EXTN], F8, kind="ExternalInput").ap()
    out = nc.dram_tensor(
        "out", [128, NACC + NCHUNK_COL], F32, kind="ExternalOutput"
    ).ap()

    with tile.TileContext(nc) as tc, ExitStack() as ctx:
        big = ctx.enter_context(tc.tile_pool(name="big", bufs=1))
        stats = ctx.enter_context(tc.tile_pool(name="stats", bufs=1))
        eop = ctx.enter_context(tc.tile_pool(name="eop", bufs=3))
        scrp = ctx.enter_context(tc.tile_pool(name="scrp", bufs=2))

        XA = big.tile([128, KT, EXTN], F8)
        COL = big.tile([128, COLW], BF16)
        OUT = stats.tile([128, NACC + NCHUNK_COL], F32)
        ones = stats.tile([128, 1], BF16)

        # Input DMAs: leads + early windows' data on sync (that ring starts
        # earliest and its completion sem gates the first matmuls); the tail
        # halves go via the scalar queue so they never gate window 0.
        nc.sync.dma_start(XA[:, 0, 0:512], xa0[:, 0:512])
        nc.sync.dma_start(XA[:, 1, 0:512], xa1[:, 0:512])
        nc.sync.dma_start(XA[:, 0, 512:4224], xa0[:, 512:4224])
        nc.sync.dma_start(XA[:, 1, 512:4224], xa1[:, 512:4224])
        for a, b in ((4224, 8192), (8192, EXTN)):
            nc.scalar.dma_start(XA[:, 0, a:b], xa0[:, a:b])
            nc.scalar.dma_start(XA[:, 1, a:b], xa1[:, a:b])

        warm = stats.tile([128, 1], F32)
        wacc = stats.tile([128, 1], F32)
        nc.vector.memset(warm[:], 0.0)
        nc.vector.memset(ones[:], 1.0)
        # Prefetch the exp table set while input DMAs stream.
        nc.scalar.activation(warm[:], warm[:], AF.Exp, accum_out=wacc[:])

        with tc.tile_pool(name="mm_psum", bufs=2, space="PSUM") as mm_psum, \
             tc.tile_pool(name="t2_psum", bufs=1, space="PSUM") as t2_psum, \
             tc.tile_pool(name="colsum_psum", bufs=1, space="PSUM") as col_psum:
            psC = col_psum.tile([128, NCHUNK_COL], F32)

            def ones_chunks(ks):
                for k in ks:
                    nc.tensor.matmul(
                        psC[:, k : k + 1],
                        lhsT=COL[:, k * 128 : (k + 1) * 128],
                        rhs=ones[:],
                        start=True,
                        stop=True,
                    )

            def mm_tile(ps, mt, lo, hi):
                # matmul chunks of <=512 cols (one PSUM bank each)
                lhsT = XA[:, :, 1024 * mt : 1024 * mt + 128]
                for off in range(0, hi - lo, 512):
                    cw = min(512, hi - lo - off)
                    xc = 1024 * mt + lo + off
                    nc.tensor.matmul(
                        ps[:, off : off + cw],
                        lhsT=lhsT,
                        rhs=XA[:, :, xc : xc + cw],
                        start=True,
                        stop=True,
                        perf_mode=DR,
                    )

            for mt in range(MT):
                c0 = 1024 * mt  # COL base for this window
                a0 = 4 * mt  # accum column base

                T0 = mm_psum.tile([128, T0W], F32, tag="T", name="T0")
                mm_tile(T0, mt, 0, T0W)
                if mt >= 1:
                    ones_chunks(range(8 * (mt - 1), 8 * mt))
                T1 = mm_psum.tile([128, T1W], F32, tag="T", name="T1")
                mm_tile(T1, mt, T0W, T0W + T1W)

                eo = eop.tile([128, WIN], BF16, tag="eo")
                scr = scrp.tile([128, 128], BF16, tag="scr")

                # ACT: exp of [0,1536) and [1536,3072) with rowsum accums
                nc.scalar.activation(
                    eo[:, 0:T0W], T0[:], AF.Exp, scale=EXP_SCALE,
                    accum_out=OUT[:, a0 : a0 + 1],
                )
                nc.scalar.activation(
                    eo[:, T0W : T0W + T1W], T1[:], AF.Exp, scale=EXP_SCALE,
                    accum_out=OUT[:, a0 + 1 : a0 + 2],
                )
                # T2 minis (1-bank ring) + DVE bitcast-exp of [3072,4224)
                for lo, hi in ((3072, 3584), (3584, 4096), (4096, WIN)):
                    T2 = t2_psum.tile([128, 512], F32, tag="U", name="T2")
                    mm_tile(T2, mt, lo, hi)
                    nc.vector.tensor_scalar(
                        eo[:, lo:hi].bitcast(I16), T2[:, 0 : hi - lo],
                        SCH_A, SCH_B, ALU.mult, ALU.add,
                    )
                # E45 first-writer fold: [3072,4096) -> COL, with rowsum accum
                nc.vector.tensor_scalar(
                    COL[:, c0 + 2944 : c0 + 3968], eo[:, 3072:4096], 1.0, 0.0,
                    ALU.mult, ALU.add, accum_out=OUT[:, a0 + 2 : a0 + 3],
                )
                # d=32 block rowsum vehicle
                nc.vector.tensor_scalar(
                    scr[:, 0:128], eo[:, 4096:4224], 1.0, 0.0,
                    ALU.mult, ALU.add, accum_out=OUT[:, a0 + 3 : a0 + 4],
                )
                # fold adds: dma-accumulate on the software-DGE queue (the
                # DMA fabric is otherwise idle); DVE takes the remainder.
                if mt == 0:
                    nc.gpsimd.dma_start(
                        COL[:, 0:DMA_W], eo[:, 128 : 128 + DMA_W]
                    )
                    nc.vector.tensor_scalar(
                        COL[:, DMA_W:2944], eo[:, 128 + DMA_W : 3072],
                        1.0, 0.0, ALU.mult, ALU.add,
                    )
                elif mt == MT - 1:
                    # last window on DVE to shorten the tail
                    nc.vector.tensor_add(
                        COL[:, c0 : c0 + 2944],
                        COL[:, c0 : c0 + 2944],
                        eo[:, 128:3072],
                    )
                else:
                    nc.gpsimd.dma_start(
                        COL[:, c0 : c0 + DMA_W],
                        eo[:, 128 : 128 + DMA_W],
                        accum_op=ALU.add,
                    )
                    nc.vector.tensor_add(
                        COL[:, c0 + DMA_W : c0 + 2944],
                        COL[:, c0 + DMA_W : c0 + 2944],
                        eo[:, 128 + DMA_W : 3072],
                    )

            # tail: H_7..H_10 = chunks 56..87
            ones_chunks(range(56, 64))
            ones_chunks(range(64, 72))
            ones_chunks(range(72, NCHUNK_COL))

            nc.vector.tensor_copy(OUT[:, NACC:], psC[:])

        nc.sync.dma_start(out, OUT[:])

    nc.compile()
    return nc


def _get_nc():
    if "nc" not in _CACHE:
        _CACHE["nc"] = _build()
    return _CACHE["nc"]


def _first_pos(y: np.ndarray) -> np.ndarray:
    y = np.asarray(y)
    uniq, first = np.unique(y, return_index=True)
    lookup = {int(v): int(f) for v, f in zip(uniq, first)}
    return np.array([lookup[int(v)] for v in y], dtype=np.int64)


def make_in_maps(x: np.ndarray, y: np.ndarray):
    x = np.asarray(x, dtype=np.float32)
    norm = np.maximum(np.sqrt((x * x).sum(axis=1, keepdims=True)), EPS)
    xn = x / norm

    fp = _first_pos(y)
    target_total = float((xn * xn[fp]).sum(dtype=np.float64) * INV_TEMP)

    f8 = mybir.dt.np(F8)
    xq = (xn * SCALE).astype(f8)  # [N, D]
    # DoubleRow transposed layout: xfT[p, s, j] = xq[j, s*128 + p]
    xfT = np.ascontiguousarray(xq.T.reshape(KT, 128, N).transpose(1, 0, 2))
    x2 = np.concatenate([xfT, xfT], axis=2)  # wrap-around halo

    in_maps = []
    for c in range(NCORES):
        off = 128 * c
        xe = x2[:, :, off : off + EXTN]
        in_maps.append({
            "xa0": np.ascontiguousarray(xe[:, 0]),
            "xa1": np.ascontiguousarray(xe[:, 1]),
        })
    return in_maps, target_total


def run(in_maps, trace=False, **kwargs):
    nc = _get_nc()
    return bass_utils.run_bass_kernel_spmd(
        nc, in_maps, core_ids=list(range(NCORES)), trace=trace, **kwargs
    )


def finish(results, target_total: float) -> np.ndarray:
    rowsum = np.zeros(N, dtype=np.float64)
    for c, r in enumerate(results):
        o = np.asarray(r["out"], dtype=np.float64)  # [128, 32+87]
        for mt in range(MT):
            base = (8 * mt + c) * 128
            rowsum[base : base + 128] += o[:, 4 * mt : 4 * mt + 4].sum(axis=1)
        colv = o[:, NACC:]  # [128, 87]; COL col = 128k + m <-> ext 128 + that
        g = (128 * c + 128 + 128 * np.arange(NCHUNK_COL)[None, :]
             + np.arange(128)[:, None]) % N
        np.add.at(rowsum, g, colv)
    lse_sum = np.log(rowsum).sum()
    return np.asarray(np.float32((lse_sum - target_total) / N))


def kernel(x: np.ndarray, y: np.ndarray) -> np.ndarray:
    in_maps, target_total = make_in_maps(x, y)
    res = run(in_maps)
    return finish(res.results, target_total)


# revision 8
# speedup vs baseline: 1.0584x; 1.0584x over previous
"""NT-Xent loss kernel for Trainium2, SPMD across 8 NeuronCores.

Strategy (symmetric/circulant decomposition, ~half the exp work), v2:
  - Host precomputes xn = x/||x|| (f32), scales by SCALE and quantizes to
    fp8-e4m3 in the DoubleRow transposed layout [128, 2, EXTN] (k = s*128+p),
    rotated per core so the device program is identical across cores.
  - sim is symmetric: only block-distances d = (jblk - iblk) mod 64 in
    {0..32} (128-wide blocks) are computed. Core c owns m-tile rows
    t = 8*mt + c; each of its 8 windows is the contiguous circulant range
    of 33 blocks = 4224 cols. lhsT for window mt is XE[:, :, 1024mt:+128].
  - exp is split across three engines per window (ext-relative cols):
      ACT:  exp([0,1408)) and exp([1408,2944)) from PSUM tiles T0/T1 with
            accum_out rowsum parts (covers the d=0 block too).
      DVE:  bitcast-exp of T2 [2944,4224): i16 = round(psum*A + B) viewed
            as bf16 ~= exp (Schraudolph, ~+-3%), 1 instr, no LUT.
      Folds (transposed/column contributions, COL coords = 1024mt+ext-128):
        Pool: add eo[128,2048) into COL[1024mt:+1920)      (window 0: copy)
        DVE:  add eo[2048,3072) into COL[1024mt+1920:+2944) (window 0: copy)
        DVE:  first-writer copy eo[3072,4096) -> COL[1024mt+2944:+3968)
              with accum_out (rowsum part), plus scratch copies of
              [2944,3072) and the d=32 block [4096,4224) with accum_out.
    No COL memset: the window-0 copies + first-writer E45 writes cover COL.
  - Column sums: ones-matmuls on the tensor engine per 128-col chunk,
    software-pipelined one window behind the folds.
  - Outputs per core: [128, 40] rowsum partials (5 accums x 8 windows) and
    [128, 87] column-sum partials. Host: rowsum assembly (np.add.at for the
    transposed parts), loss = (sum log(rowsum) - target)/N.
"""

import sys

sys.path.insert(0, "/opt/trn_rl_repo")

from contextlib import ExitStack

import numpy as np

import concourse.bass as bass
import concourse.tile as tile
from concourse import bacc, bass_utils, mybir

F32 = mybir.dt.float32
F8 = mybir.dt.float8e4
BF16 = mybir.dt.bfloat16
I16 = mybir.dt.int16
AF = mybir.ActivationFunctionType
ALU = mybir.AluOpType
DR = mybir.MatmulPerfMode.DoubleRow

N, D = 8192, 256
NCORES = 8
SHARD = N // NCORES  # 1024 rows per core
MT = SHARD // 128  # 8 windows per core
KT = 2  # two 128-row k-subtiles (D=256), packed via DoubleRow
NBLK = 33
WIN = NBLK * 128  # 4224
EXTN = (MT - 1) * 1024 + WIN  # 11392 extended cols
COLW = (MT - 1) * 1024 + (WIN - 128) - 128  # 11136 col accumulator width
NCHUNK_COL = COLW // 128  # 87
NACC = 4 * MT  # 32 rowsum accumulator columns

TEMP = 0.5
INV_TEMP = 1.0 / TEMP
EPS = 1e-8
SCALE = 8.0  # fp8 pre-scale on xn entries
EXP_SCALE = INV_TEMP / (SCALE * SCALE)

# Schraudolph bitcast-exp: i16 = round(psum*SCH_A + SCH_B); bits viewed as
# bf16 give ~exp(psum*EXP_SCALE).  CADJ centers the sawtooth error.
LN2 = 0.6931471805599453
CADJ = 0.043
SCH_A = EXP_SCALE * 128.0 / LN2
SCH_B = 128.0 * (127.0 - CADJ)

# ext-relative region boundaries (PSUM tiles / engine ownership)
T0W, T1W = 1536, 1536  # ACT psum tiles [0,1536) [1536,3072)
DMA_W = 1920  # dma-accum fold piece: eo[128, 2048) -> COL[1024mt, +1920)

_CACHE = {}


def _build():
    nc = bacc.Bacc("TRN2", target_bir_lowering=False, debug=False, num_devices=NCORES)

    xa0 = nc.dram_tensor("xa0", [128, EXTN], F8, kind="ExternalInput").ap()
    xa1 = nc.dram_tensor("xa1", [128, EXTN], F8, kind="ExternalInput").ap()
    out = nc.dram_tensor(
        "out", [128, NACC + NCHUNK_COL], F32, kind="ExternalOutput"
    ).ap()

    with tile.TileContext(nc) as tc, ExitStack() as ctx:
        big = ctx.enter_context(tc.tile_pool(name="big", bufs=1))
        stats = ctx.enter_context(tc.tile_pool(name="stats", bufs=1))
        eop = ctx.enter_context(tc.tile_pool(name="eop", bufs=3))
        scrp = ctx.enter_context(tc.tile_pool(name="scrp", bufs=2))

        XA = big.tile([128, KT, EXTN], F8)
        COL = big.tile([128, COLW], BF16)
        OUT = stats.tile([128, NACC + NCHUNK_COL], F32)
        ones = stats.tile([128, 1], BF16)

        # Input DMA leads only -- the bulk chunks are issued after window
        # 0's first matmuls (program order), so the conservative DMA-sem
        # consolidation cannot gate the first matmul on them.
        nc.sync.dma_start(XA[:, 0, 0:512], xa0[:, 0:512])
        nc.sync.dma_start(XA[:, 1, 0:512], xa1[:, 0:512])

        warm = stats.tile([128, 1], F32)
        wacc = stats.tile([128, 1], F32)
        nc.vector.memset(warm[:], 0.0)
        nc.vector.memset(ones[:], 1.0)
        # Prefetch the exp table set while input DMAs stream.
        nc.scalar.activation(warm[:], warm[:], AF.Exp, accum_out=wacc[:])

        with tc.tile_pool(name="mm_psum", bufs=2, space="PSUM") as mm_psum, \
             tc.tile_pool(name="t2_psum", bufs=1, space="PSUM") as t2_psum, \
             tc.tile_pool(name="colsum_psum", bufs=1, space="PSUM") as col_psum:
            psC = col_psum.tile([128, NCHUNK_COL], F32)

            def ones_chunks(ks):
                for k in ks:
                    nc.tensor.matmul(
                        psC[:, k : k + 1],
                        lhsT=COL[:, k * 128 : (k + 1) * 128],
                        rhs=ones[:],
                        start=True,
                        stop=True,
                    )

            def mm_tile(ps, mt, lo, hi, po=0):
                # matmul chunks of <=512 cols (one PSUM bank each)
                lhsT = XA[:, :, 1024 * mt : 1024 * mt + 128]
                for off in range(0, hi - lo, 512):
                    cw = min(512, hi - lo - off)
                    xc = 1024 * mt + lo + off
                    nc.tensor.matmul(
                        ps[:, po + off : po + off + cw],
                        lhsT=lhsT,
                        rhs=XA[:, :, xc : xc + cw],
                        start=True,
                        stop=True,
                        perf_mode=DR,
                    )

            for mt in range(MT):
                c0 = 1024 * mt  # COL base for this window
                a0 = 4 * mt  # accum column base

                T0 = mm_psum.tile([128, T0W], F32, tag="T", name="T0")
                if mt == 0:
                    mm_tile(T0, 0, 0, 512)
                    # bulk input now that the first matmul is already issued
                    nc.sync.dma_start(XA[:, 0, 512:4224], xa0[:, 512:4224])
                    nc.sync.dma_start(XA[:, 1, 512:4224], xa1[:, 512:4224])
                    for a, b in ((4224, 8192), (8192, EXTN)):
                        nc.scalar.dma_start(XA[:, 0, a:b], xa0[:, a:b])
                        nc.scalar.dma_start(XA[:, 1, a:b], xa1[:, a:b])
                    mm_tile(T0, 0, 512, T0W, po=512)
                else:
                    mm_tile(T0, mt, 0, T0W)
                    ones_chunks(range(8 * (mt - 1), 8 * mt))
                T1 = mm_psum.tile([128, T1W], F32, tag="T", name="T1")
                mm_tile(T1, mt, T0W, T0W + T1W)

                eo = eop.tile([128, WIN], BF16, tag="eo")
                scr = scrp.tile([128, 128], BF16, tag="scr")

                # ACT: exp of [0,1536) and [1536,3072) with rowsum accums
                nc.scalar.activation(
                    eo[:, 0:T0W], T0[:], AF.Exp, scale=EXP_SCALE,
                    accum_out=OUT[:, a0 : a0 + 1],
                )
                nc.scalar.activation(
                    eo[:, T0W : T0W + T1W], T1[:], AF.Exp, scale=EXP_SCALE,
                    accum_out=OUT[:, a0 + 1 : a0 + 2],
                )
                # T2 minis (1-bank ring) + DVE bitcast-exp of [3072,4224)
                for lo, hi in ((3072, 3584), (3584, 4096), (4096, WIN)):
                    T2 = t2_psum.tile([128, 512], F32, tag="U", name="T2")
                    mm_tile(T2, mt, lo, hi)
                    nc.vector.tensor_scalar(
                        eo[:, lo:hi].bitcast(I16), T2[:, 0 : hi - lo],
                        SCH_A, SCH_B, ALU.mult, ALU.add,
                    )
                # E45 first-writer fold: [3072,4096) -> COL, with rowsum accum
                nc.vector.tensor_scalar(
                    COL[:, c0 + 2944 : c0 + 3968], eo[:, 3072:4096], 1.0, 0.0,
                    ALU.mult, ALU.add, accum_out=OUT[:, a0 + 2 : a0 + 3],
                )
                # d=32 block rowsum vehicle
                nc.vector.tensor_scalar(
                    scr[:, 0:128], eo[:, 4096:4224], 1.0, 0.0,
                    ALU.mult, ALU.add, accum_out=OUT[:, a0 + 3 : a0 + 4],
                )
                # fold adds: dma-accumulate on the software-DGE queue (the
                # DMA fabric is otherwise idle); DVE takes the remainder.
                if mt == 0:
                    nc.gpsimd.dma_start(
                        COL[:, 0:DMA_W], eo[:, 128 : 128 + DMA_W]
                    )
                    nc.vector.tensor_scalar(
                        COL[:, DMA_W:2944], eo[:, 128 + DMA_W : 3072],
                        1.0, 0.0, ALU.mult, ALU.add,
                    )
                elif mt >= MT - 2:
                    # last windows on DVE to shorten the fold-dma tail
                    nc.vector.tensor_add(
                        COL[:, c0 : c0 + 2944],
                        COL[:, c0 : c0 + 2944],
                        eo[:, 128:3072],
                    )
                else:
                    nc.gpsimd.dma_start(
                        COL[:, c0 : c0 + DMA_W],
                        eo[:, 128 : 128 + DMA_W],
                        accum_op=ALU.add,
                    )
                    nc.vector.tensor_add(
                        COL[:, c0 + DMA_W : c0 + 2944],
                        COL[:, c0 + DMA_W : c0 + 2944],
                        eo[:, 128 + DMA_W : 3072],
                    )

            # tail: H_7..H_10 = chunks 56..87
            ones_chunks(range(56, 64))
            ones_chunks(range(64, 72))
            ones_chunks(range(72, NCHUNK_COL))

            nc.vector.tensor_copy(OUT[:, NACC:], psC[:])

        nc.sync.dma_start(out, OUT[:])

    nc.compile()
    return nc


def _get_nc():
    if "nc" not in _CACHE:
        _CACHE["nc"] = _build()
    return _CACHE["nc"]


def _first_pos(y: np.ndarray) -> np.ndarray:
    y = np.asarray(y)
    uniq, first = np.unique(y, return_index=True)
    lookup = {int(v): int(f) for v, f in zip(uniq, first)}
    return np.array([lookup[int(v)] for v in y], dtype=np.int64)


def make_in_maps(x: np.ndarray, y: np.ndarray):
    x = np.asarray(x, dtype=np.float32)
    norm = np.maximum(np.sqrt((x * x).sum(axis=1, keepdims=True)), EPS)
    xn = x / norm

    fp = _first_pos(y)
    target_total = float((xn * xn[fp]).sum(dtype=np.float64) * INV_TEMP)

    f8 = mybir.dt.np(F8)
    xq = (xn * SCALE).astype(f8)  # [N, D]
    # DoubleRow transposed layout: xfT[p, s, j] = xq[j, s*128 + p]
    xfT = np.ascontiguousarray(xq.T.reshape(KT, 128, N).transpose(1, 0, 2))
    x2 = np.concatenate([xfT, xfT], axis=2)  # wrap-around halo

    in_maps = []
    for c in range(NCORES):
        off = 128 * c
        xe = x2[:, :, off : off + EXTN]
        in_maps.append({
            "xa0": np.ascontiguousarray(xe[:, 0]),
            "xa1": np.ascontiguousarray(xe[:, 1]),
        })
    return in_maps, target_total


def run(in_maps, trace=False, **kwargs):
    nc = _get_nc()
    return bass_utils.run_bass_kernel_spmd(
        nc, in_maps, core_ids=list(range(NCORES)), trace=trace, **kwargs
    )


def finish(results, target_total: float) -> np.ndarray:
    rowsum = np.zeros(N, dtype=np.float64)
    for c, r in enumerate(results):
        o = np.asarray(r["out"], dtype=np.float64)  # [128, 32+87]
        for mt in range(MT):
            base = (8 * mt + c) * 128
            rowsum[base : base + 128] += o[:, 4 * mt : 4 * mt + 4].sum(axis=1)
        colv = o[:, NACC:]  # [128, 87]; COL col = 128k + m <-> ext 128 + that
        g = (128 * c + 128 + 128 * np.arange(NCHUNK_COL)[None, :]
             + np.arange(128)[:, None]) % N
        np.add.at(rowsum, g, colv)
    lse_sum = np.log(rowsum).sum()
    return np.asarray(np.float32((lse_sum - target_total) / N))


def kernel(x: np.ndarray, y: np.ndarray) -> np.ndarray:
    in_maps, target_total = make_in_maps(x, y)
    res = run(in_maps)
    return finish(res.results, target_total)


# revision 9
# speedup vs baseline: 1.3308x; 1.2574x over previous
"""NT-Xent loss kernel for Trainium2, SPMD across 8 NeuronCores.

Strategy (symmetric/circulant decomposition, ~half the exp work):
  - Host precomputes xn = x/||x|| (f32), scales by SCALE and quantizes to
    fp8-e4m3 in the DoubleRow transposed layout [128, 2, N] (k = s*128+p).
  - sim is symmetric, so only block-distances d = (jblk - iblk) mod 64 in
    {0..32} (128-wide blocks) are computed. Rows are owned interleaved:
    core c owns m-tile rows t = 8*mt + c. Each m-tile processes the
    contiguous circulant window of 33 blocks = 4224 cols. Entries with
    d in {1..31} also serve as the transposed entries via column sums;
    d=0 (diagonal block) and d=32 (self-paired distance, computed twice
    globally) contribute row sums only.
  - Host ships, per core, a rotated+extended matrix xe (ext col j <->
    global col (128c + j) mod 8192) so the device program is identical
    across cores, plus the core's own 8 m-tile rows for the stationary
    operand.
  - Device: fp8 DoubleRow matmuls -> fused exp+row-sum (ACT accum_out),
    exp output (bf16) folded into a column-sum accumulator on the vector
    engine; partition-reduction of column sums via ones-matmuls on the
    tensor engine. Outputs per core: [128, 25] row-sum partials and
    [128, 87] column-sum partials.
  - Host finishes: rowsum_total, loss = (sum log(rowsum) - target)/N.
"""

import sys

sys.path.insert(0, "/opt/trn_rl_repo")

from contextlib import ExitStack

import numpy as np

import concourse.bass as bass
import concourse.tile as tile
from concourse import bacc, bass_utils, mybir

F32 = mybir.dt.float32
F8 = mybir.dt.float8e4
BF16 = mybir.dt.bfloat16
AF = mybir.ActivationFunctionType
ALU = mybir.AluOpType
DR = mybir.MatmulPerfMode.DoubleRow

N, D = 8192, 256
NCORES = 8
SHARD = N // NCORES  # 1024 rows per core
MT = SHARD // 128  # 8 m-tiles per core
KT = 2  # two 128-row k-subtiles (D=256), packed via DoubleRow
NBLK = 33  # circulant window: block distances 0..32
WIN = NBLK * 128  # 4224 cols per m-tile window
EXTN = (MT - 1) * 1024 + WIN  # 11392 extended cols
SPAN = WIN // 3  # 1408: ACT span (3 PSUM banks)
NSPAN = 3
SPANS = ((0, SPAN), (SPAN, 2 * SPAN), (2 * SPAN, WIN))
SPANS0 = ((0, 512), (512, SPAN), (SPAN, 2 * SPAN), (2 * SPAN, WIN))
SPANS7 = SPANS
NACC = len(SPANS0) + (MT - 2) * len(SPANS) + len(SPANS7)  # 25 accum columns
COLW = (MT - 1) * 1024 + (WIN - 128) - 128  # 11136 col-sum accumulator width
NCHUNK_COL = COLW // 128  # 87 ones-matmul chunks
TEMP = 0.5
INV_TEMP = 1.0 / TEMP
EPS = 1e-8
SCALE = 8.0  # fp8 pre-scale on xn entries
EXP_SCALE = INV_TEMP / (SCALE * SCALE)

_CACHE = {}


def _build():
    nc = bacc.Bacc("TRN2", target_bir_lowering=False, debug=False, num_devices=NCORES)

    # merged input: [mt0 lhsT (128) | XE[0:512) | XL rest (896) | XE rest]
    # so one small leading DMA carries the first matmul span's operands
    xa = nc.dram_tensor("xa", [128, KT, SHARD + EXTN], F8, kind="ExternalInput").ap()
    out = nc.dram_tensor(
        "out", [128, NACC + NCHUNK_COL], F32, kind="ExternalOutput"
    ).ap()

    with tile.TileContext(nc) as tc, ExitStack() as ctx:
        big = ctx.enter_context(tc.tile_pool(name="big", bufs=1))
        io = ctx.enter_context(tc.tile_pool(name="io", bufs=1))
        stats = ctx.enter_context(tc.tile_pool(name="stats", bufs=1))
        eop = ctx.enter_context(tc.tile_pool(name="eop", bufs=3))

        XA = big.tile([128, KT, SHARD + EXTN], F8)
        COL = big.tile([128, COLW], BF16)
        OUT = stats.tile([128, NACC + NCHUNK_COL], F32)
        ones = stats.tile([128, 1], BF16)

        # Input DMAs alternate across the two hardware queues; the scalar
        # queue's issues come first on that engine so its transfers start
        # early, with the exp table load filling the gap before the first
        # real exp. Leading 640-col sync chunk carries the first matmul
        # span's operands.
        for a, b in ((640, 2432), (3840, 5248), (7040, 8832), (10624, SHARD + EXTN)):
            nc.scalar.dma_start(XA[:, :, a:b], xa[:, :, a:b])
        for a, b in ((0, 640), (2432, 3840), (5248, 7040), (8832, 10624)):
            nc.sync.dma_start(XA[:, :, a:b], xa[:, :, a:b])

        # Small memsets first, big COL memset last — COL is not needed
        # until the first fold.
        warm = stats.tile([128, 1], F32)
        wacc = stats.tile([128, 1], F32)
        nc.vector.memset(warm[:], 0.0)
        nc.vector.memset(ones[:], 1.0)
        # Prefetch the exp table set while input DMAs stream.
        nc.scalar.activation(warm[:], warm[:], AF.Exp, accum_out=wacc[:])
        nc.vector.memset(COL[:].bitcast(F32), 0.0)

        with tc.tile_pool(name="mm_psum", bufs=2, space="PSUM") as mm_psum, \
             tc.tile_pool(name="colsum_psum", bufs=1, space="PSUM") as col_psum:
            psC = col_psum.tile([128, NCHUNK_COL], F32)

            # m-tile 0's first span is split so the exp stream starts as
            # soon as the first 512-col DMA chunk lands.
            acc_idx = [0]

            def xcol(j):
                # XE column j -> merged-layout column
                return 128 + j if j < 512 else 1536 + (j - 512)

            def main_mt(mt):
                lo = 0 if mt == 0 else 640 + (mt - 1) * 128
                lhsT = XA[:, :, lo : lo + 128]
                eo = eop.tile([128, WIN], BF16, tag="eo")
                base = mt * 1024
                spans = SPANS0 if mt == 0 else (SPANS7 if mt == MT - 1 else SPANS)
                for s0, s1 in spans:
                    w = s1 - s0
                    ps = mm_psum.tile([128, SPAN], F32)
                    for off in range(0, w, 512):
                        cw = min(512, w - off)
                        xc = xcol(base + s0 + off)
                        nc.tensor.matmul(
                            ps[:, off : off + cw],
                            lhsT=lhsT,
                            rhs=XA[:, :, xc : xc + cw],
                            start=True,
                            stop=True,
                            perf_mode=DR,
                        )
                    ai = acc_idx[0]
                    acc_idx[0] += 1
                    nc.scalar.activation(
                        eo[:, s0:s1],
                        ps[:, 0:w],
                        AF.Exp,
                        scale=EXP_SCALE,
                        accum_out=OUT[:, ai : ai + 1],
                    )
                    # col-sum-eligible part (d=0 and d=32 blocks excluded)
                    f0, f1 = max(s0, 128), min(s1, WIN - 128)
                    if f0 < f1:
                        c = base + f0 - 128
                        nc.vector.tensor_add(
                            COL[:, c : c + (f1 - f0)],
                            COL[:, c : c + (f1 - f0)],
                            eo[:, f0:f1],
                        )

            def ones_chunks(ks):
                for k in ks:
                    nc.tensor.matmul(
                        psC[:, k : k + 1],
                        lhsT=COL[:, k * 128 : (k + 1) * 128],
                        rhs=ones[:],
                        start=True,
                        stop=True,
                    )

            # software pipeline: chunks [8mt, 8mt+8) are final once
            # fold(mt, span0) has run (earlier windows' folds precede it in
            # DVE program order); issue them on the PE one m-tile behind.
            for mt in range(MT):
                main_mt(mt)
                if mt >= 1:
                    ones_chunks(range((mt - 1) * 8, mt * 8))
            # window 7 finalizes chunks 56..65 (its span-0 region), then
            # 66..76 (span 1) and 77..86 (span 2)
            ones_chunks(range(56, 66))
            ones_chunks(range(66, 77))
            ones_chunks(range(77, NCHUNK_COL))

            nc.vector.tensor_copy(OUT[:, NACC:], psC[:])

        nc.sync.dma_start(out, OUT[:])

    nc.compile()
    return nc


def _get_nc():
    if "nc" not in _CACHE:
        _CACHE["nc"] = _build()
    return _CACHE["nc"]


def _first_pos(y: np.ndarray) -> np.ndarray:
    """first_pos[i] = first index j with y[j] == y[i]."""
    y = np.asarray(y)
    uniq, first = np.unique(y, return_index=True)
    lookup = {int(v): int(f) for v, f in zip(uniq, first)}
    return np.array([lookup[int(v)] for v in y], dtype=np.int64)


def make_in_maps(x: np.ndarray, y: np.ndarray):
    x = np.asarray(x, dtype=np.float32)
    norm = np.maximum(np.sqrt((x * x).sum(axis=1, keepdims=True)), EPS)
    xn = x / norm

    # target term (exact, f32): sum_i sim[i, first_pos_i]
    fp = _first_pos(y)
    target_total = float((xn * xn[fp]).sum(dtype=np.float64) * INV_TEMP)

    f8 = mybir.dt.np(F8)
    xq = (xn * SCALE).astype(f8)  # [N, D]
    # DoubleRow transposed layout: xfT[p, s, j] = xq[j, s*128 + p]
    xfT = np.ascontiguousarray(xq.T.reshape(KT, 128, N).transpose(1, 0, 2))
    x2 = np.concatenate([xfT, xfT], axis=2)  # wrap-around halo

    in_maps = []
    for c in range(NCORES):
        off = 128 * c
        xe = x2[:, :, off : off + EXTN]
        xl = np.empty((128, KT, SHARD), dtype=f8)
        for mt in range(MT):
            r = (8 * mt + c) * 128
            xl[:, :, mt * 128 : (mt + 1) * 128] = xfT[:, :, r : r + 128]
        xa = np.concatenate(
            [xl[:, :, 0:128], xe[:, :, 0:512], xl[:, :, 128:], xe[:, :, 512:]],
            axis=2,
        )
        in_maps.append({"xa": np.ascontiguousarray(xa)})
    return in_maps, target_total


def run(in_maps, trace=False, **kwargs):
    nc = _get_nc()
    return bass_utils.run_bass_kernel_spmd(
        nc, in_maps, core_ids=list(range(NCORES)), trace=trace, **kwargs
    )


def finish(results, target_total: float) -> np.ndarray:
    rowsum = np.zeros(N, dtype=np.float64)
    for c, r in enumerate(results):
        o = np.asarray(r["out"], dtype=np.float64)  # [128, 26+87]
        a = 0
        for mt in range(MT):
            spans = SPANS0 if mt == 0 else (SPANS7 if mt == MT - 1 else SPANS)
            b = a + len(spans)
            base = (8 * mt + c) * 128
            rowsum[base : base + 128] += o[:, a:b].sum(axis=1)
            a = b
        colv = o[:, NACC:]  # [128, 87]; ext col = 128 + 128k + m
        g = (128 * c + 128 + 128 * np.arange(NCHUNK_COL)[None, :]
             + np.arange(128)[:, None]) % N
        np.add.at(rowsum, g, colv)
    lse_sum = np.log(rowsum).sum()
    return np.asarray(np.float32((lse_sum - target_total) / N))


def kernel(x: np.ndarray, y: np.ndarray) -> np.ndarray:
    in_maps, target_total = make_in_maps(x, y)
    res = run(in_maps)
    return finish(res.results, target_total)

